# revision 7
# baseline (speedup 1.0000x reference)
"""BirthDeathAttention kernel for 8 Trainium2 NeuronCores.

Math note: in the reference, both `persistence_bias` ([1,H,1,1]) and
`importance_weights[:, None, :, None] * 0.1` ([B,1,N,1]) are constant along
the softmax (key) axis, so they cancel exactly inside the softmax.  The
module is therefore plain multi-head attention + output projection.

Sharding (per the tensor-parallel hint): core = (batch b, head-group g),
b in {0,1}, g in {0..3}, each core handling 4 of the 16 heads for one batch
element.  Each core computes a partial output projection (its heads' slice
of W_proj rows); the host sums the 4 partials per batch and adds b_proj.

Per-core device pipeline (all matmuls bf16, accumulation fp32).  The
schedule is paced by the ScalarE exp stream (the hard floor: B*H*N*N/8 =
16.8M exps/core); everything else hides under it:
  A) qk^T = W_qk^T x^T    -> q^T,k^T channel-major [512, 2048]
  B) v    = x W_v          -> position-major [2048, 256]
  C) per wave (head-pair, 512-query block), unit = (key-tile, head):
       S^T units = k^T q    row-tiled pairs (two heads concurrent on the
                            64-contraction PE array halves)
       E = exp(SCALE*S^T)   ScalarE, one instruction per THREE units
                            ([128,1536] spanning 3 PSUM banks) to amortize
                            the ~293ns/instruction ACT overhead
       U = v^T E^T          column-tiled pairs: head a -> PSUM rows 0-63,
                            head b -> rows 64-127 (two heads concurrent)
       denom = 1^T E^T      four concurrent M=1 column-tiled chains
                            (positions 0,32 head a / 64,96 head b), folded
                            pairwise on DVE
       O^T = U * (1/denom)  evacuate U to SBUF, normalize in place
  E) partial_out = O W_p   -> [2048, 1024], DMA to DRAM
"""

import sys

if "/opt/trn_rl_repo" not in sys.path:
    sys.path.insert(0, "/opt/trn_rl_repo")

import numpy as np
import ml_dtypes

import concourse.bass as bass
import concourse.mybir as mybir
import concourse.tile as tile
from concourse.bass_utils import run_bass_kernel_spmd

DIM = 1024
N = 2048
B = 2
HEADS = 16
HEAD_DIM = 64
SCALE = HEAD_DIM ** -0.5
HPG = 4          # heads per group (per core)
GC = HPG * HEAD_DIM  # channels per core = 256
BF16 = mybir.dt.bfloat16
F32 = mybir.dt.float32

KT = DIM // 128      # 8 contraction tiles over model dim
NB = N // 512        # 4 query blocks
NKT = N // 128       # 16 key tiles

# exp instruction groups per wave: 10 x 3 units + 1 x 2 units (unit =
# (key-tile, head) = [128,512] PSUM bank written by one S matmul)
GROUPS = [(3 * i, 3) for i in range(10)] + [(30, 2)]


def _split_multi_waits(nc, max_waits=1):
    """The walrus build in this container accepts at most one sync-wait per
    instruction.  Hoist extra waits onto single-wait NOPs inserted just
    before the instruction in its engine's program order (instructions on
    one engine execute in order, so an AND of waits on one instruction is
    equivalent to a chain of single-wait NOPs followed by the rest)."""
    uid = [0]
    for f in nc.m.functions:
        for bb in f.blocks:
            insts = bb.instructions
            new = []
            changed = False
            for inst in insts:
                si = inst.sync_info
                if si is not None and len(si.on_wait) > max_waits:
                    waits = list(si.on_wait)
                    for w in waits[:-max_waits]:
                        nop = mybir.InstNoOp(
                            name=f"I-splitw-{uid[0]}", ins=[], outs=[])
                        uid[0] += 1
                        nop.engine = inst.engine
                        nop.sync_info = mybir.SyncInfo(
                            on_wait=[w], on_update=[])
                        new.append(nop)
                    si.on_wait = waits[-max_waits:]
                    inst.sync_info = si
                    changed = True
                new.append(inst)
            if changed:
                bb.instructions = new


def build_core_kernel() -> bass.Bass:
    nc = bass.Bass()
    xT = nc.declare_dram_parameter("xT", [DIM, N], BF16, isOutput=False)
    wqk = nc.declare_dram_parameter("wqk", [DIM, 2 * GC], BF16, isOutput=False)
    wv = nc.declare_dram_parameter("wv", [DIM, GC], BF16, isOutput=False)
    wp = nc.declare_dram_parameter("wp", [GC, DIM], BF16, isOutput=False)
    out = nc.declare_dram_parameter("out", [N, DIM], BF16, isOutput=True)

    xT_r = xT.rearrange("(kt p) n -> p kt n", p=128)
    wqk_r = wqk.rearrange("(kt p) c -> p kt c", p=128)
    wv_r = wv.rearrange("(kt p) c -> p kt c", p=128)
    wp_r = wp.rearrange("(pair p) c -> p pair c", p=128)

    with tile.TileContext(nc) as tc:
        from contextlib import ExitStack

        with ExitStack() as ctx:
            consts = ctx.enter_context(tc.tile_pool(name="consts", bufs=1))
            sbuf = ctx.enter_context(tc.tile_pool(name="sbuf", bufs=1))

            # --- resident SBUF tensors -------------------------------------
            xT_sb = sbuf.tile([128, KT, N], BF16, tag="xT")
            wqk_sb = consts.tile([128, KT, 2 * GC], BF16, tag="wqk")
            wv_sb = consts.tile([128, KT, GC], BF16, tag="wv")
            wp_sb = consts.tile([128, 2, DIM], BF16, tag="wp")
            ones_sb = consts.tile([128, 1], BF16, tag="ones")
            qk_sb = sbuf.tile([128, 4, N], BF16, tag="qk")
            v_sb = sbuf.tile([128, NKT, GC], BF16, tag="v")
            o_sb = sbuf.tile([128, 2, N], BF16, tag="o")

            nc.vector.memset(ones_sb[:], 1.0)

            # Few large DMAs on two issue rings: one big transfer is split
            # across all 16 SDMA engines, so fewer+larger beats many chunks.
            # Critical path (first q/k chains) = wqk + first xT quarter.
            nc.sync.dma_start(out=wqk_sb[:, :, :], in_=wqk_r[:, :, :])
            nc.sync.dma_start(out=xT_sb[:, :, 0:512], in_=xT_r[:, :, 0:512])
            nc.gpsimd.dma_start(
                out=xT_sb[:, :, 512:1024], in_=xT_r[:, :, 512:1024])
            nc.gpsimd.dma_start(
                out=xT_sb[:, :, 1024:2048], in_=xT_r[:, :, 1024:2048])
            nc.gpsimd.dma_start(out=wv_sb[:], in_=wv_r[:])
            nc.gpsimd.dma_start(out=wp_sb[:], in_=wp_r[:])

            # Helper emitters ---------------------------------------------
            def a_chain(pool, ct, nb):
                acc = pool.tile([128, 512], F32, tag="psA")
                for kt in range(KT):
                    nc.tensor.matmul(
                        acc[:],
                        lhsT=wqk_sb[:, kt, ct * 128:(ct + 1) * 128],
                        rhs=xT_sb[:, kt, nb * 512:(nb + 1) * 512],
                        start=(kt == 0),
                        stop=(kt == KT - 1),
                    )
                nc.vector.tensor_copy(
                    qk_sb[:, ct, nb * 512:(nb + 1) * 512], acc[:]
                )

            def b_chain(pool, nt):
                acc = pool.tile([128, GC], F32, tag="psA")
                for kt in range(KT):
                    nc.tensor.matmul(
                        acc[:],
                        lhsT=xT_sb[:, kt, nt * 128:(nt + 1) * 128],
                        rhs=wv_sb[:, kt, :],
                        start=(kt == 0),
                        stop=(kt == KT - 1),
                    )
                nc.vector.tensor_copy(v_sb[:, nt, :], acc[:])

            def s_groups(psS, e_t, pair, nqb, g0, g1):
                # S^T units + exp, three units per ACT instruction
                qt = qk_sb[:, pair, :]
                kt_sb = qk_sb[:, 2 + pair, :]
                for gi in range(g0, g1):
                    u0, cnt = GROUPS[gi]
                    st = psS.tile([128, 1536], F32, tag="st")
                    for j in range(cnt):
                        u = u0 + j
                        nkt, hh = divmod(u, 2)
                        nc.tensor.matmul(
                            st[:, j * 512:(j + 1) * 512],
                            lhsT=kt_sb[
                                hh * 64:(hh + 1) * 64,
                                nkt * 128:(nkt + 1) * 128,
                            ],
                            rhs=qt[
                                hh * 64:(hh + 1) * 64,
                                nqb * 512:(nqb + 1) * 512,
                            ],
                            start=True,
                            stop=True,
                        )
                    nc.scalar.activation(
                        e_t[:, u0 * 512:(u0 + cnt) * 512],
                        st[:, 0:cnt * 512],
                        mybir.ActivationFunctionType.Exp,
                        scale=SCALE,
                    )

            def u_block(psU, pair, nqb, e_t):
                # column-tiled pair: head a -> PSUM rows 0-63, head b ->
                # rows 64-127; both stream concurrently per key tile
                u_t = psU.tile([128, 512], F32, tag="u")
                for nkt in range(NKT):
                    for hh in range(2):
                        h = pair * 2 + hh
                        nc.tensor.matmul(
                            u_t[hh * 64:(hh + 1) * 64, :],
                            lhsT=v_sb[:, nkt, h * 64:(h + 1) * 64],
                            rhs=e_t[
                                :, (2 * nkt + hh) * 512:
                                (2 * nkt + hh + 1) * 512],
                            start=(nkt == 0),
                            stop=(nkt == NKT - 1),
                        )
                # evacuate unnormalized U to SBUF (frees the PSUM slot for
                # the projection stage); normalized in place later
                nc.vector.tensor_copy(
                    o_sb[0:64, pair, nqb * 512:(nqb + 1) * 512], u_t[0:64, :])
                nc.vector.tensor_copy(
                    o_sb[64:128, pair, nqb * 512:(nqb + 1) * 512],
                    u_t[64:128, :])

            def dn_block(psD, pair, nqb, e_t):
                # softmax denominators: four concurrent M=1 column-tiled
                # accumulation chains (head a at PSUM rows 0/32 over key
                # halves, head b at 64/96); j-major so consecutive matmuls
                # sit on different column tiles and overlap
                dn_t = psD.tile([128, 512], F32, tag="dn")
                for j in range(8):
                    for hh in range(2):
                        for half in range(2):
                            nkt = half * 8 + j
                            pos = hh * 64 + half * 32
                            nc.tensor.matmul(
                                dn_t[pos:pos + 1, :],
                                lhsT=ones_sb[:, 0:1],
                                rhs=e_t[
                                    :, (2 * nkt + hh) * 512:
                                    (2 * nkt + hh + 1) * 512],
                                start=(j == 0),
                                stop=(j == 7),
                                tile_position=(0, pos),
                            )
                return dn_t

            def norm(rpool, rdram, dn_t, pair, nqb):
                r_x = rpool.tile([33, 512], F32, tag="rx")
                r_in = rpool.tile([33, 512], F32, tag="rin")
                r_t = rpool.tile([33, 512], F32, tag="r")
                rr_t = rpool.tile([128, 512], F32, tag="rr")
                r_dr = rdram.tile([2, 512], F32, tag="rdr")
                # fold the per-key-half partial sums (a tensor-tensor op may
                # read at most one PSUM operand, so stage one side in SBUF)
                nc.vector.tensor_copy(r_x[0:1, :], dn_t[32:33, :])
                nc.vector.tensor_copy(r_x[32:33, :], dn_t[96:97, :])
                nc.vector.tensor_add(
                    r_in[0:1, :], dn_t[0:1, :], r_x[0:1, :])
                nc.vector.tensor_add(
                    r_in[32:33, :], dn_t[64:65, :], r_x[32:33, :])
                # one reciprocal covers both heads' sum rows (rows 1..31
                # are unused garbage)
                nc.vector.reciprocal(r_t[0:33, :], r_in[0:33, :])
                for hh in range(2):
                    nc.sync.dma_start(
                        out=r_dr[hh:hh + 1, :],
                        in_=r_t[hh * 32:hh * 32 + 1, :],
                    )
                    nc.sync.dma_start(
                        out=rr_t[hh * 64:(hh + 1) * 64, :],
                        in_=r_dr[hh:hh + 1, :].to_broadcast([64, 512]),
                    )
                # normalize in place (bf16 O^T = bf16(U) * recip)
                sl = o_sb[:, pair, nqb * 512:(nqb + 1) * 512]
                nc.vector.tensor_mul(sl[0:64, :], sl[0:64, :], rr_t[0:64, :])
                nc.vector.tensor_mul(
                    sl[64:128, :], sl[64:128, :], rr_t[64:128, :])

            def e_block(psU, psD, opool, qb):
                # partial projection for query block qb (both pairs
                # normalized); nh chains alternate between two PSUM slots
                for mt in range(qb * 4, qb * 4 + 4):
                    ot = opool.tile([128, DIM], BF16, tag="ot")
                    acc0 = psU.tile([128, 512], F32, tag="u")
                    acc1 = psD.tile([128, 512], F32, tag="dn")
                    for pair in range(2):
                        for nh, acc in ((0, acc0), (1, acc1)):
                            nc.tensor.matmul(
                                acc[:],
                                lhsT=o_sb[:, pair, mt * 128:(mt + 1) * 128],
                                rhs=wp_sb[:, pair, nh * 512:(nh + 1) * 512],
                                start=(pair == 0),
                                stop=(pair == 1),
                            )
                    nc.vector.tensor_copy(ot[:, 0:512], acc0[:])
                    nc.vector.tensor_copy(ot[:, 512:1024], acc1[:])
                    nc.sync.dma_start(
                        out=out[mt * 128:(mt + 1) * 128, :], in_=ot[:]
                    )

            # --- software-pipelined schedule -------------------------------
            iters = [(nqb, pair) for nqb in range(NB) for pair in range(2)]
            with (
                tc.tile_pool(name="psS", bufs=2, space="PSUM") as psS,
                tc.tile_pool(name="epool", bufs=3) as epool,
                tc.tile_pool(name="rpool", bufs=2) as rpool,
                tc.tile_pool(name="rdram", bufs=2, space="DRAM") as rdram,
                tc.tile_pool(name="opool", bufs=2) as opool,
            ):
                e_list = []
                psA_ctx = ExitStack()
                psA = psA_ctx.enter_context(
                    tc.tile_pool(name="psA", bufs=2, space="PSUM"))
                # Prologue: wave 0's S/exp groups unlock incrementally as
                # the k-projection chains land; remaining projections fill
                # the PE while those exps drain.
                a_chain(psA, 2, 0)
                a_chain(psA, 0, 0)
                e_t0 = epool.tile([128, NKT * 1024], BF16, tag="e")
                e_list.append(e_t0)
                s_groups(psS, e_list[0], 0, 0, 0, 2)     # units 0..5
                a_chain(psA, 2, 1)
                s_groups(psS, e_list[0], 0, 0, 2, 5)     # units 6..14
                a_chain(psA, 2, 2)
                s_groups(psS, e_list[0], 0, 0, 5, 8)     # units 15..23
                a_chain(psA, 2, 3)
                s_groups(psS, e_list[0], 0, 0, 8, 11)    # units 24..31
                for nb in range(NB):
                    a_chain(psA, 3, nb)
                a_chain(psA, 1, 0)
                e_t1 = epool.tile([128, NKT * 1024], BF16, tag="e")
                e_list.append(e_t1)
                s_groups(psS, e_list[1], 1, 0, 0, 11)    # wave 1 = (0,1)
                a_chain(psA, 0, 1)
                a_chain(psA, 1, 1)
                for nt in range(NKT):
                    b_chain(psA, nt)
                a_chain(psA, 0, 2)
                a_chain(psA, 1, 2)
                a_chain(psA, 0, 3)
                a_chain(psA, 1, 3)
                psA_ctx.close()

                with (
                    tc.tile_pool(name="psU", bufs=1, space="PSUM") as psU,
                    tc.tile_pool(name="psD", bufs=1, space="PSUM") as psD,
                ):
                    for k, (nqb, pair) in enumerate(iters):
                        u_block(psU, pair, nqb, e_list[k])
                        dn_t = dn_block(psD, pair, nqb, e_list[k])
                        if k % 2 == 0 and k >= 2:
                            e_block(psU, psD, opool, k // 2 - 1)
                        if k + 2 < len(iters):
                            nq2, p2 = iters[k + 2]
                            e_tn = epool.tile([128, NKT * 1024], BF16, tag="e")
                            e_list.append(e_tn)
                            s_groups(psS, e_list[k + 2], p2, nq2, 0, 11)
                        norm(rpool, rdram, dn_t, pair, nqb)
                    e_block(psU, psD, opool, 3)

    _split_multi_waits(nc)
    return nc


_NC_CACHE = None


def _get_nc():
    global _NC_CACHE
    if _NC_CACHE is None:
        _NC_CACHE = build_core_kernel()
    return _NC_CACHE


def kernel(x, importance_weights, W_qkv, W_proj, b_proj, persistence_bias,
           _results_hook=None):
    x = np.asarray(x)
    W_qkv = np.asarray(W_qkv, dtype=np.float32)
    W_proj = np.asarray(W_proj, dtype=np.float32)
    b_proj = np.asarray(b_proj, dtype=np.float32)

    bf = ml_dtypes.bfloat16
    Q = W_qkv[:, 0:DIM]
    K = W_qkv[:, DIM:2 * DIM]
    V = W_qkv[:, 2 * DIM:3 * DIM]

    in_maps = []
    for core in range(8):
        b, g = divmod(core, 4)
        sl = slice(g * GC, (g + 1) * GC)
        in_maps.append({
            "xT": np.ascontiguousarray(x[b].T).astype(bf),
            "wqk": np.ascontiguousarray(
                np.concatenate([Q[:, sl], K[:, sl]], axis=1)).astype(bf),
            "wv": np.ascontiguousarray(V[:, sl]).astype(bf),
            "wp": np.ascontiguousarray(W_proj[sl, :]).astype(bf),
        })

    nc = _get_nc()
    res = run_bass_kernel_spmd(nc, in_maps, list(range(8)))
    if _results_hook is not None:
        _results_hook(res)

    out = np.zeros((B, N, DIM), dtype=np.float32)
    for core in range(8):
        b = core // 4
        out[b] += res.results[core]["out"].astype(np.float32)
    out += b_proj[None, None, :]
    return out


# revision 12
# speedup vs baseline: 1.0175x; 1.0175x over previous
"""BirthDeathAttention kernel for 8 Trainium2 NeuronCores.

Math note: in the reference, both `persistence_bias` ([1,H,1,1]) and
`importance_weights[:, None, :, None] * 0.1` ([B,1,N,1]) are constant along
the softmax (key) axis, so they cancel exactly inside the softmax.  The
module is therefore plain multi-head attention + output projection.

Sharding (per the tensor-parallel hint): core = (batch b, head-group g),
b in {0,1}, g in {0..3}, each core handling 4 of the 16 heads for one batch
element.  Each core computes a partial output projection (its heads' slice
of W_proj rows); the host sums the 4 partials per batch and adds b_proj.

Per-core device pipeline (all matmuls bf16, accumulation fp32).  The
schedule is paced by the ScalarE exp stream (the hard floor: B*H*N*N/8 =
16.8M exps/core); everything else hides under it:
  A) qk^T = W_qk^T x^T    -> q^T,k^T channel-major [512, 2048]
  B) v    = x W_v          -> position-major [2048, 256]
  C) per wave (head-pair, 512-query block), unit = (key-tile, head):
       S^T units = k^T q    row-tiled pairs (two heads concurrent on the
                            64-contraction PE array halves)
       E = exp(SCALE*S^T)   ScalarE, one instruction per THREE units
                            ([128,1536] spanning 3 PSUM banks) to amortize
                            the ~293ns/instruction ACT overhead
       U = v^T E^T          column-tiled pairs: head a -> PSUM rows 0-63,
                            head b -> rows 64-127 (two heads concurrent)
       denom = 1^T E^T      four concurrent M=1 column-tiled chains
                            (positions 0,32 head a / 64,96 head b), folded
                            pairwise on DVE
       O^T = U * (1/denom)  evacuate U to SBUF, normalize in place
  E) partial_out = O W_p   -> [2048, 1024], DMA to DRAM
"""

import sys

if "/opt/trn_rl_repo" not in sys.path:
    sys.path.insert(0, "/opt/trn_rl_repo")

import numpy as np
import ml_dtypes

import concourse.bass as bass
import concourse.mybir as mybir
import concourse.tile as tile
from concourse.bass_utils import run_bass_kernel_spmd

DIM = 1024
N = 2048
B = 2
HEADS = 16
HEAD_DIM = 64
SCALE = HEAD_DIM ** -0.5
HPG = 4          # heads per group (per core)
GC = HPG * HEAD_DIM  # channels per core = 256
BF16 = mybir.dt.bfloat16
F32 = mybir.dt.float32

KT = DIM // 128      # 8 contraction tiles over model dim
NB = N // 512        # 4 query blocks
NKT = N // 128       # 16 key tiles

# exp instruction groups per wave: 10 x 3 units + 1 x 2 units (unit =
# (key-tile, head) = [128,512] PSUM bank written by one S matmul)
GROUPS = [(3 * i, 3) for i in range(10)] + [(30, 2)]


def _split_multi_waits(nc, max_waits=1):
    """The walrus build in this container accepts at most one sync-wait per
    instruction.  Hoist extra waits onto single-wait NOPs inserted just
    before the instruction in its engine's program order (instructions on
    one engine execute in order, so an AND of waits on one instruction is
    equivalent to a chain of single-wait NOPs followed by the rest)."""
    uid = [0]
    for f in nc.m.functions:
        for bb in f.blocks:
            insts = bb.instructions
            new = []
            changed = False
            for inst in insts:
                si = inst.sync_info
                if si is not None and len(si.on_wait) > max_waits:
                    waits = list(si.on_wait)
                    for w in waits[:-max_waits]:
                        nop = mybir.InstNoOp(
                            name=f"I-splitw-{uid[0]}", ins=[], outs=[])
                        uid[0] += 1
                        nop.engine = inst.engine
                        nop.sync_info = mybir.SyncInfo(
                            on_wait=[w], on_update=[])
                        new.append(nop)
                    si.on_wait = waits[-max_waits:]
                    inst.sync_info = si
                    changed = True
                new.append(inst)
            if changed:
                bb.instructions = new


def build_core_kernel() -> bass.Bass:
    nc = bass.Bass()
    xT = nc.declare_dram_parameter("xT", [DIM, N], BF16, isOutput=False)
    wqk = nc.declare_dram_parameter("wqk", [DIM, 2 * GC], BF16, isOutput=False)
    wv = nc.declare_dram_parameter("wv", [DIM, GC], BF16, isOutput=False)
    wp = nc.declare_dram_parameter("wp", [GC, DIM], BF16, isOutput=False)
    out = nc.declare_dram_parameter("out", [N, DIM], BF16, isOutput=True)

    xT_r = xT.rearrange("(kt p) n -> p kt n", p=128)
    wqk_r = wqk.rearrange("(kt p) c -> p kt c", p=128)
    wv_r = wv.rearrange("(kt p) c -> p kt c", p=128)
    wp_r = wp.rearrange("(pair p) c -> p pair c", p=128)

    with tile.TileContext(nc) as tc:
        from contextlib import ExitStack

        with ExitStack() as ctx:
            consts = ctx.enter_context(tc.tile_pool(name="consts", bufs=1))
            sbuf = ctx.enter_context(tc.tile_pool(name="sbuf", bufs=1))

            # --- resident SBUF tensors -------------------------------------
            xT_sb = sbuf.tile([128, KT, N], BF16, tag="xT")
            wqk_sb = consts.tile([128, KT, 2 * GC], BF16, tag="wqk")
            wv_sb = consts.tile([128, KT, GC], BF16, tag="wv")
            wp_sb = consts.tile([128, 2, DIM], BF16, tag="wp")
            ones_sb = consts.tile([128, 1], BF16, tag="ones")
            qk_sb = sbuf.tile([128, 4, N], BF16, tag="qk")
            v_sb = sbuf.tile([128, NKT, GC], BF16, tag="v")
            o_sb = sbuf.tile([128, 2, N], BF16, tag="o")

            nc.vector.memset(ones_sb[:], 1.0)

            # Few large DMAs (one transfer is split across all 16 SDMA
            # engines), all on the sync ring (gpsimd-issued DMAs pay a slow
            # software descriptor-generation loop).  Order matches consumer
            # order: wqk + first xT quarter unlock the first q/k chains.
            nc.sync.dma_start(out=wqk_sb[:, :, :], in_=wqk_r[:, :, :])
            for q in range(4):
                nc.sync.dma_start(
                    out=xT_sb[:, :, q * 512:(q + 1) * 512],
                    in_=xT_r[:, :, q * 512:(q + 1) * 512])
            nc.sync.dma_start(out=wv_sb[:], in_=wv_r[:])
            nc.sync.dma_start(out=wp_sb[:], in_=wp_r[:])

            # Helper emitters ---------------------------------------------
            def a_chain(pool, ct, nb):
                acc = pool.tile([128, 512], F32, tag="psA")
                for kt in range(KT):
                    nc.tensor.matmul(
                        acc[:],
                        lhsT=wqk_sb[:, kt, ct * 128:(ct + 1) * 128],
                        rhs=xT_sb[:, kt, nb * 512:(nb + 1) * 512],
                        start=(kt == 0),
                        stop=(kt == KT - 1),
                    )
                nc.vector.tensor_copy(
                    qk_sb[:, ct, nb * 512:(nb + 1) * 512], acc[:]
                )

            def b_chain(pool, nt):
                acc = pool.tile([128, GC], F32, tag="psA")
                for kt in range(KT):
                    nc.tensor.matmul(
                        acc[:],
                        lhsT=xT_sb[:, kt, nt * 128:(nt + 1) * 128],
                        rhs=wv_sb[:, kt, :],
                        start=(kt == 0),
                        stop=(kt == KT - 1),
                    )
                nc.vector.tensor_copy(v_sb[:, nt, :], acc[:])

            def s_groups(psS, e_t, pair, nqb, g0, g1):
                # S^T units + exp, three units per ACT instruction
                qt = qk_sb[:, pair, :]
                kt_sb = qk_sb[:, 2 + pair, :]
                for gi in range(g0, g1):
                    u0, cnt = GROUPS[gi]
                    st = psS.tile([128, 1536], F32, tag="st")
                    for j in range(cnt):
                        u = u0 + j
                        nkt, hh = divmod(u, 2)
                        nc.tensor.matmul(
                            st[:, j * 512:(j + 1) * 512],
                            lhsT=kt_sb[
                                hh * 64:(hh + 1) * 64,
                                nkt * 128:(nkt + 1) * 128,
                            ],
                            rhs=qt[
                                hh * 64:(hh + 1) * 64,
                                nqb * 512:(nqb + 1) * 512,
                            ],
                            start=True,
                            stop=True,
                        )
                    nc.scalar.activation(
                        e_t[:, u0 * 512:(u0 + cnt) * 512],
                        st[:, 0:cnt * 512],
                        mybir.ActivationFunctionType.Exp,
                        scale=SCALE,
                    )

            def u_block(psU, pair, nqb, e_t):
                # column-tiled pair: head a -> PSUM rows 0-63, head b ->
                # rows 64-127; both stream concurrently per key tile
                u_t = psU.tile([128, 512], F32, tag="u")
                for nkt in range(NKT):
                    for hh in range(2):
                        h = pair * 2 + hh
                        nc.tensor.matmul(
                            u_t[hh * 64:(hh + 1) * 64, :],
                            lhsT=v_sb[:, nkt, h * 64:(h + 1) * 64],
                            rhs=e_t[
                                :, (2 * nkt + hh) * 512:
                                (2 * nkt + hh + 1) * 512],
                            start=(nkt == 0),
                            stop=(nkt == NKT - 1),
                        )
                # evacuate unnormalized U to SBUF (frees the PSUM slot for
                # the projection stage); normalized in place later
                nc.vector.tensor_copy(
                    o_sb[0:64, pair, nqb * 512:(nqb + 1) * 512], u_t[0:64, :])
                nc.vector.tensor_copy(
                    o_sb[64:128, pair, nqb * 512:(nqb + 1) * 512],
                    u_t[64:128, :])

            def dn_block(psD, pair, nqb, e_t):
                # softmax denominators: four concurrent M=1 column-tiled
                # accumulation chains (head a at PSUM rows 0/32 over key
                # halves, head b at 64/96); j-major so consecutive matmuls
                # sit on different column tiles and overlap
                dn_t = psD.tile([128, 512], F32, tag="dn")
                for j in range(8):
                    for hh in range(2):
                        for half in range(2):
                            nkt = half * 8 + j
                            pos = hh * 64 + half * 32
                            nc.tensor.matmul(
                                dn_t[pos:pos + 1, :],
                                lhsT=ones_sb[:, 0:1],
                                rhs=e_t[
                                    :, (2 * nkt + hh) * 512:
                                    (2 * nkt + hh + 1) * 512],
                                start=(j == 0),
                                stop=(j == 7),
                                tile_position=(0, pos),
                            )
                return dn_t

            def norm(rpool, rdram, dn_t, pair, nqb):
                r_x = rpool.tile([33, 512], F32, tag="rx")
                r_in = rpool.tile([33, 512], F32, tag="rin")
                r_t = rpool.tile([33, 512], F32, tag="r")
                rr_t = rpool.tile([128, 512], F32, tag="rr")
                r_dr = rdram.tile([2, 512], F32, tag="rdr")
                # fold the per-key-half partial sums (a tensor-tensor op may
                # read at most one PSUM operand, so stage one side in SBUF)
                nc.vector.tensor_copy(r_x[0:1, :], dn_t[32:33, :])
                nc.vector.tensor_copy(r_x[32:33, :], dn_t[96:97, :])
                nc.vector.tensor_add(
                    r_in[0:1, :], dn_t[0:1, :], r_x[0:1, :])
                nc.vector.tensor_add(
                    r_in[32:33, :], dn_t[64:65, :], r_x[32:33, :])
                # one reciprocal covers both heads' sum rows (rows 1..31
                # are unused garbage)
                nc.vector.reciprocal(r_t[0:33, :], r_in[0:33, :])
                for hh in range(2):
                    nc.sync.dma_start(
                        out=r_dr[hh:hh + 1, :],
                        in_=r_t[hh * 32:hh * 32 + 1, :],
                    )
                    nc.sync.dma_start(
                        out=rr_t[hh * 64:(hh + 1) * 64, :],
                        in_=r_dr[hh:hh + 1, :].to_broadcast([64, 512]),
                    )
                # normalize in place (bf16 O^T = bf16(U) * recip)
                sl = o_sb[:, pair, nqb * 512:(nqb + 1) * 512]
                nc.vector.tensor_mul(sl[0:64, :], sl[0:64, :], rr_t[0:64, :])
                nc.vector.tensor_mul(
                    sl[64:128, :], sl[64:128, :], rr_t[64:128, :])

            def e_block(psU, psD, opool, mt0, mt1):
                # partial projection rows [mt0,mt1) (both pairs of that
                # query block normalized); nh chains alternate PSUM slots
                for mt in range(mt0, mt1):
                    ot = opool.tile([128, DIM], BF16, tag="ot")
                    acc0 = psU.tile([128, 512], F32, tag="u")
                    acc1 = psD.tile([128, 512], F32, tag="dn")
                    for pair in range(2):
                        for nh, acc in ((0, acc0), (1, acc1)):
                            nc.tensor.matmul(
                                acc[:],
                                lhsT=o_sb[:, pair, mt * 128:(mt + 1) * 128],
                                rhs=wp_sb[:, pair, nh * 512:(nh + 1) * 512],
                                start=(pair == 0),
                                stop=(pair == 1),
                            )
                    nc.vector.tensor_copy(ot[:, 0:512], acc0[:])
                    nc.vector.tensor_copy(ot[:, 512:1024], acc1[:])
                    nc.sync.dma_start(
                        out=out[mt * 128:(mt + 1) * 128, :], in_=ot[:]
                    )

            # --- software-pipelined schedule -------------------------------
            iters = [(nqb, pair) for nqb in range(NB) for pair in range(2)]
            with (
                tc.tile_pool(name="psS", bufs=2, space="PSUM") as psS,
                tc.tile_pool(name="epool", bufs=3) as epool,
                tc.tile_pool(name="rpool", bufs=2) as rpool,
                tc.tile_pool(name="rdram", bufs=2, space="DRAM") as rdram,
                tc.tile_pool(name="opool", bufs=2) as opool,
            ):
                e_list = []
                psA_ctx = ExitStack()
                psA = psA_ctx.enter_context(
                    tc.tile_pool(name="psA", bufs=2, space="PSUM"))
                # Prologue: wave 0's S/exp groups unlock incrementally as
                # the k-projection chains land; remaining projections fill
                # the PE while those exps drain.
                a_chain(psA, 2, 0)
                a_chain(psA, 0, 0)
                e_t0 = epool.tile([128, NKT * 1024], BF16, tag="e")
                e_list.append(e_t0)
                s_groups(psS, e_list[0], 0, 0, 0, 2)     # units 0..5
                a_chain(psA, 2, 1)
                s_groups(psS, e_list[0], 0, 0, 2, 5)     # units 6..14
                a_chain(psA, 2, 2)
                s_groups(psS, e_list[0], 0, 0, 5, 8)     # units 15..23
                a_chain(psA, 2, 3)
                s_groups(psS, e_list[0], 0, 0, 8, 11)    # units 24..31
                for nb in range(NB):
                    a_chain(psA, 3, nb)
                a_chain(psA, 1, 0)
                e_t1 = epool.tile([128, NKT * 1024], BF16, tag="e")
                e_list.append(e_t1)
                s_groups(psS, e_list[1], 1, 0, 0, 11)    # wave 1 = (0,1)
                # waves 2 and 3 go into the PE program BEFORE the b-chains:
                # their S matmuls dribble behind the exp stream's PSUM
                # rotation, and the b/a chains fill the gaps — emitting them
                # after the chains would starve the ScalarE for ~15us at the
                # prologue/steady-state boundary.
                a_chain(psA, 0, 1)
                a_chain(psA, 1, 1)
                e_t2 = epool.tile([128, NKT * 1024], BF16, tag="e")
                e_list.append(e_t2)
                s_groups(psS, e_list[2], 0, 1, 0, 11)    # wave 2 = (1,0)
                for nt in range(NKT):
                    b_chain(psA, nt)
                a_chain(psA, 0, 2)
                a_chain(psA, 1, 2)
                a_chain(psA, 0, 3)
                a_chain(psA, 1, 3)
                psA_ctx.close()

                with (
                    tc.tile_pool(name="psU", bufs=1, space="PSUM") as psU,
                    tc.tile_pool(name="psD", bufs=1, space="PSUM") as psD,
                ):
                    for k, (nqb, pair) in enumerate(iters):
                        if k + 3 < len(iters):
                            nq2, p2 = iters[k + 3]
                            e_tn = epool.tile([128, NKT * 1024], BF16, tag="e")
                            e_list.append(e_tn)
                            s_groups(psS, e_list[k + 3], p2, nq2, 0, 11)
                        u_block(psU, pair, nqb, e_list[k])
                        dn_t = dn_block(psD, pair, nqb, e_list[k])
                        norm(rpool, rdram, dn_t, pair, nqb)
                        # projection rows for the previous fully-normalized
                        # query block, two row-tiles per iteration
                        if k >= 2:
                            qb = (k - 2) // 2
                            half = (k - 2) % 2
                            e_block(psU, psD, opool,
                                    qb * 4 + half * 2, qb * 4 + half * 2 + 2)
                    e_block(psU, psD, opool, 12, 16)

    _split_multi_waits(nc)
    return nc


_NC_CACHE = None


def _get_nc():
    global _NC_CACHE
    if _NC_CACHE is None:
        _NC_CACHE = build_core_kernel()
    return _NC_CACHE


def kernel(x, importance_weights, W_qkv, W_proj, b_proj, persistence_bias,
           _results_hook=None):
    x = np.asarray(x)
    W_qkv = np.asarray(W_qkv, dtype=np.float32)
    W_proj = np.asarray(W_proj, dtype=np.float32)
    b_proj = np.asarray(b_proj, dtype=np.float32)

    bf = ml_dtypes.bfloat16
    Q = W_qkv[:, 0:DIM]
    K = W_qkv[:, DIM:2 * DIM]
    V = W_qkv[:, 2 * DIM:3 * DIM]

    in_maps = []
    for core in range(8):
        b, g = divmod(core, 4)
        sl = slice(g * GC, (g + 1) * GC)
        in_maps.append({
            "xT": np.ascontiguousarray(x[b].T).astype(bf),
            "wqk": np.ascontiguousarray(
                np.concatenate([Q[:, sl], K[:, sl]], axis=1)).astype(bf),
            "wv": np.ascontiguousarray(V[:, sl]).astype(bf),
            "wp": np.ascontiguousarray(W_proj[sl, :]).astype(bf),
        })

    nc = _get_nc()
    res = run_bass_kernel_spmd(nc, in_maps, list(range(8)))
    if _results_hook is not None:
        _results_hook(res)

    out = np.zeros((B, N, DIM), dtype=np.float32)
    for core in range(8):
        b = core // 4
        out[b] += res.results[core]["out"].astype(np.float32)
    out += b_proj[None, None, :]
    return out


# revision 20
# speedup vs baseline: 1.0303x; 1.0126x over previous
"""BirthDeathAttention kernel for 8 Trainium2 NeuronCores.

Math note: in the reference, both `persistence_bias` ([1,H,1,1]) and
`importance_weights[:, None, :, None] * 0.1` ([B,1,N,1]) are constant along
the softmax (key) axis, so they cancel exactly inside the softmax.  The
module is therefore plain multi-head attention + output projection.

Sharding (per the tensor-parallel hint): core = (batch b, head-group g),
b in {0,1}, g in {0..3}, each core handling 4 of the 16 heads for one batch
element.  Each core computes a partial output projection (its heads' slice
of W_proj rows); the host sums the 4 partials per batch and adds b_proj.

Per-core device pipeline (all matmuls bf16, accumulation fp32).  The
schedule is paced by the ScalarE exp stream (the hard floor: B*H*N*N/8 =
16.8M exps/core); everything else hides under it:
  A) qk^T = W_qk^T x^T    -> q^T,k^T channel-major [512, 2048]
  B) v    = x W_v          -> position-major [2048, 256]
  C) per wave (head-pair, 512-query block), unit = (key-tile, head):
       S^T units = k^T q    row-tiled pairs (two heads concurrent on the
                            64-contraction PE array halves)
       E = exp(SCALE*S^T)   ScalarE, one instruction per THREE units
                            ([128,1536] spanning 3 PSUM banks) to amortize
                            the ~293ns/instruction ACT overhead
       U = v^T E^T          column-tiled pairs: head a -> PSUM rows 0-63,
                            head b -> rows 64-127 (two heads concurrent)
       denom = 1^T E^T      four concurrent M=1 column-tiled chains
                            (positions 0,32 head a / 64,96 head b), folded
                            pairwise on DVE
       O^T = U * (1/denom)  evacuate U to SBUF, normalize in place
  E) partial_out = O W_p   -> [2048, 1024], DMA to DRAM
"""

import sys

if "/opt/trn_rl_repo" not in sys.path:
    sys.path.insert(0, "/opt/trn_rl_repo")

import numpy as np
import ml_dtypes

import concourse.bass as bass
import concourse.mybir as mybir
import concourse.tile as tile
from concourse.bass_utils import run_bass_kernel_spmd

DIM = 1024
N = 2048
B = 2
HEADS = 16
HEAD_DIM = 64
SCALE = HEAD_DIM ** -0.5
HPG = 4          # heads per group (per core)
GC = HPG * HEAD_DIM  # channels per core = 256
BF16 = mybir.dt.bfloat16
F32 = mybir.dt.float32

KT = DIM // 128      # 8 contraction tiles over model dim
NB = N // 512        # 4 query blocks
NKT = N // 128       # 16 key tiles

# exp instruction groups per wave: 10 x 3 units + 1 x 2 units (unit =
# (key-tile, head) = [128,512] PSUM bank written by one S matmul)
GROUPS = [(3 * i, 3) for i in range(10)] + [(30, 2)]


def _split_multi_waits(nc, max_waits=1):
    """The walrus build in this container accepts at most one sync-wait per
    instruction.  Hoist extra waits onto single-wait NOPs inserted just
    before the instruction in its engine's program order (instructions on
    one engine execute in order, so an AND of waits on one instruction is
    equivalent to a chain of single-wait NOPs followed by the rest)."""
    uid = [0]
    for f in nc.m.functions:
        for bb in f.blocks:
            insts = bb.instructions
            new = []
            changed = False
            for inst in insts:
                si = inst.sync_info
                if si is not None and len(si.on_wait) > max_waits:
                    waits = list(si.on_wait)
                    for w in waits[:-max_waits]:
                        nop = mybir.InstNoOp(
                            name=f"I-splitw-{uid[0]}", ins=[], outs=[])
                        uid[0] += 1
                        nop.engine = inst.engine
                        nop.sync_info = mybir.SyncInfo(
                            on_wait=[w], on_update=[])
                        new.append(nop)
                    si.on_wait = waits[-max_waits:]
                    inst.sync_info = si
                    changed = True
                new.append(inst)
            if changed:
                bb.instructions = new


def build_core_kernel() -> bass.Bass:
    nc = bass.Bass()
    xT = nc.declare_dram_parameter("xT", [DIM, N], BF16, isOutput=False)
    wqk = nc.declare_dram_parameter("wqk", [DIM, 2 * GC], BF16, isOutput=False)
    wv = nc.declare_dram_parameter("wv", [DIM, GC], BF16, isOutput=False)
    wp = nc.declare_dram_parameter("wp", [GC, DIM], BF16, isOutput=False)
    out = nc.declare_dram_parameter("out", [N, DIM], BF16, isOutput=True)

    xT_r = xT.rearrange("(kt p) n -> p kt n", p=128)
    wqk_r = wqk.rearrange("(kt p) c -> p kt c", p=128)
    wv_r = wv.rearrange("(kt p) c -> p kt c", p=128)
    wp_r = wp.rearrange("(pair p) c -> p pair c", p=128)

    with tile.TileContext(nc) as tc:
        from contextlib import ExitStack

        with ExitStack() as ctx:
            consts = ctx.enter_context(tc.tile_pool(name="consts", bufs=1))
            sbuf = ctx.enter_context(tc.tile_pool(name="sbuf", bufs=1))

            # --- resident SBUF tensors -------------------------------------
            xT_sb = sbuf.tile([128, KT, N], BF16, tag="xT")
            wqk_sb = consts.tile([128, KT, 2 * GC], BF16, tag="wqk")
            wv_sb = consts.tile([128, KT, GC], BF16, tag="wv")
            wp_sb = consts.tile([128, 2, DIM], BF16, tag="wp")
            ones_sb = consts.tile([128, 1], BF16, tag="ones")
            qk_sb = sbuf.tile([128, 4, N], BF16, tag="qk")
            v_sb = sbuf.tile([128, NKT, GC], BF16, tag="v")
            o_sb = sbuf.tile([128, 2, N], BF16, tag="o")

            nc.vector.memset(ones_sb[:], 1.0)

            # Few large DMAs (one transfer is split across all 16 SDMA
            # engines), all on the sync ring (gpsimd-issued DMAs pay a slow
            # software descriptor-generation loop).  Order matches consumer
            # order: wqk + first xT quarter unlock the first q/k chains.
            nc.sync.dma_start(out=wqk_sb[:, :, :], in_=wqk_r[:, :, :])
            for q in range(4):
                nc.sync.dma_start(
                    out=xT_sb[:, :, q * 512:(q + 1) * 512],
                    in_=xT_r[:, :, q * 512:(q + 1) * 512])
            nc.sync.dma_start(out=wv_sb[:], in_=wv_r[:])
            nc.sync.dma_start(out=wp_sb[:], in_=wp_r[:])

            # Helper emitters ---------------------------------------------
            def a_chain(pool, ct, nb):
                acc = pool.tile([128, 512], F32, tag="psA")
                for kt in range(KT):
                    nc.tensor.matmul(
                        acc[:],
                        lhsT=wqk_sb[:, kt, ct * 128:(ct + 1) * 128],
                        rhs=xT_sb[:, kt, nb * 512:(nb + 1) * 512],
                        start=(kt == 0),
                        stop=(kt == KT - 1),
                    )
                nc.vector.tensor_copy(
                    qk_sb[:, ct, nb * 512:(nb + 1) * 512], acc[:]
                )

            def b_chain(pool, nt):
                acc = pool.tile([128, GC], F32, tag="psA")
                for kt in range(KT):
                    nc.tensor.matmul(
                        acc[:],
                        lhsT=xT_sb[:, kt, nt * 128:(nt + 1) * 128],
                        rhs=wv_sb[:, kt, :],
                        start=(kt == 0),
                        stop=(kt == KT - 1),
                    )
                nc.vector.tensor_copy(v_sb[:, nt, :], acc[:])

            def s_groups(psS, e_t, pair, nqb, g0, g1):
                # S^T units + exp, three units per ACT instruction.  High
                # priority: this is the chain that feeds the ScalarE exp
                # stream (the pacing engine) — the scheduler must prefer
                # these over projection/b-chain fill work whenever the PSUM
                # slot rotation unblocks them.
                with tc.high_priority():
                    _s_groups(psS, e_t, pair, nqb, g0, g1)

            def _s_groups(psS, e_t, pair, nqb, g0, g1):
                qt = qk_sb[:, pair, :]
                kt_sb = qk_sb[:, 2 + pair, :]
                for gi in range(g0, g1):
                    u0, cnt = GROUPS[gi]
                    st = psS.tile([128, 1536], F32, tag="st")
                    for j in range(cnt):
                        u = u0 + j
                        nkt, hh = divmod(u, 2)
                        nc.tensor.matmul(
                            st[:, j * 512:(j + 1) * 512],
                            lhsT=kt_sb[
                                hh * 64:(hh + 1) * 64,
                                nkt * 128:(nkt + 1) * 128,
                            ],
                            rhs=qt[
                                hh * 64:(hh + 1) * 64,
                                nqb * 512:(nqb + 1) * 512,
                            ],
                            start=True,
                            stop=True,
                        )
                    nc.scalar.activation(
                        e_t[:, u0 * 512:(u0 + cnt) * 512],
                        st[:, 0:cnt * 512],
                        mybir.ActivationFunctionType.Exp,
                        scale=SCALE,
                    )

            def u_block(psU, pair, nqb, e_t):
                # column-tiled pair: head a -> PSUM rows 0-63, head b ->
                # rows 64-127; both stream concurrently per key tile
                u_t = psU.tile([128, 512], F32, tag="u")
                for nkt in range(NKT):
                    for hh in range(2):
                        h = pair * 2 + hh
                        nc.tensor.matmul(
                            u_t[hh * 64:(hh + 1) * 64, :],
                            lhsT=v_sb[:, nkt, h * 64:(h + 1) * 64],
                            rhs=e_t[
                                :, (2 * nkt + hh) * 512:
                                (2 * nkt + hh + 1) * 512],
                            start=(nkt == 0),
                            stop=(nkt == NKT - 1),
                        )
                # evacuate unnormalized U to SBUF (frees the PSUM slot for
                # the projection stage); normalized in place later
                nc.vector.tensor_copy(
                    o_sb[:, pair, nqb * 512:(nqb + 1) * 512], u_t[:])

            def dn_block(psD, pair, nqb, e_t):
                # softmax denominators: four concurrent M=1 column-tiled
                # accumulation chains (head a at PSUM rows 0/32 over key
                # halves, head b at 64/96); j-major so consecutive matmuls
                # sit on different column tiles and overlap
                dn_t = psD.tile([128, 512], F32, tag="dn")
                for j in range(8):
                    for hh in range(2):
                        for half in range(2):
                            nkt = half * 8 + j
                            # head a chains at rows {0,64}, head b at
                            # {32,96}: the fold is then one shifted [33]-row
                            # copy + one add instead of four row ops
                            pos = hh * 32 + half * 64
                            nc.tensor.matmul(
                                dn_t[pos:pos + 1, :],
                                lhsT=ones_sb[:, 0:1],
                                rhs=e_t[
                                    :, (2 * nkt + hh) * 512:
                                    (2 * nkt + hh + 1) * 512],
                                start=(j == 0),
                                stop=(j == 7),
                                tile_position=(0, pos),
                            )
                return dn_t

            def norm(rpool, rdram, dn_t, pair, nqb):
                r_x = rpool.tile([33, 512], F32, tag="rx")
                r_in = rpool.tile([33, 512], F32, tag="rin")
                r_t = rpool.tile([33, 512], F32, tag="r")
                rr_t = rpool.tile([128, 512], F32, tag="rr")
                r_dr = rdram.tile([2, 512], F32, tag="rdr")
                # fold the per-key-half partial sums: rows 64..96 shifted
                # down by 64 land the second-half chains on their heads'
                # rows (a tensor-tensor op may read at most one PSUM
                # operand, so stage the shifted copy in SBUF).  Rows other
                # than 0 and 32 are unused garbage throughout.
                nc.vector.tensor_copy(r_x[0:33, :], dn_t[64:97, :])
                nc.vector.tensor_add(
                    r_in[0:33, :], dn_t[0:33, :], r_x[0:33, :])
                nc.vector.reciprocal(r_t[0:33, :], r_in[0:33, :])
                for hh in range(2):
                    nc.sync.dma_start(
                        out=r_dr[hh:hh + 1, :],
                        in_=r_t[hh * 32:hh * 32 + 1, :],
                    )
                    nc.sync.dma_start(
                        out=rr_t[hh * 64:(hh + 1) * 64, :],
                        in_=r_dr[hh:hh + 1, :].to_broadcast([64, 512]),
                    )
                # normalize in place (bf16 O^T = bf16(U) * recip); rr_t
                # holds head a's reciprocal broadcast across rows 0-63 and
                # head b's across 64-127, so one op covers both
                sl = o_sb[:, pair, nqb * 512:(nqb + 1) * 512]
                nc.vector.tensor_mul(sl[:, :], sl[:, :], rr_t[:, :])

            def e_block(psU, psD, opool, mt0, mt1):
                # partial projection rows [mt0,mt1) (both pairs of that
                # query block normalized); nh chains alternate PSUM slots
                for mt in range(mt0, mt1):
                    ot = opool.tile([128, DIM], BF16, tag="ot")
                    acc0 = psU.tile([128, 512], F32, tag="u")
                    acc1 = psD.tile([128, 512], F32, tag="dn")
                    for pair in range(2):
                        for nh, acc in ((0, acc0), (1, acc1)):
                            nc.tensor.matmul(
                                acc[:],
                                lhsT=o_sb[:, pair, mt * 128:(mt + 1) * 128],
                                rhs=wp_sb[:, pair, nh * 512:(nh + 1) * 512],
                                start=(pair == 0),
                                stop=(pair == 1),
                            )
                    nc.vector.tensor_copy(ot[:, 0:512], acc0[:])
                    nc.vector.tensor_copy(ot[:, 512:1024], acc1[:])
                    # gpsimd ring: keeps the latency-critical reciprocal
                    # round-trip DMAs unqueued on the sync ring
                    nc.gpsimd.dma_start(
                        out=out[mt * 128:(mt + 1) * 128, :], in_=ot[:]
                    )

            # --- software-pipelined schedule -------------------------------
            iters = [(nqb, pair) for nqb in range(NB) for pair in range(2)]
            with (
                tc.tile_pool(name="psS", bufs=2, space="PSUM") as psS,
                tc.tile_pool(name="epool", bufs=3) as epool,
                tc.tile_pool(name="rpool", bufs=2) as rpool,
                tc.tile_pool(name="rdram", bufs=2, space="DRAM") as rdram,
                tc.tile_pool(name="opool", bufs=2) as opool,
            ):
                e_list = []
                psA_ctx = ExitStack()
                psA = psA_ctx.enter_context(
                    tc.tile_pool(name="psA", bufs=2, space="PSUM"))
                # Prologue: wave 0's S/exp groups unlock incrementally as
                # the k-projection chains land; remaining projections fill
                # the PE while those exps drain.
                a_chain(psA, 2, 0)
                a_chain(psA, 0, 0)
                e_t0 = epool.tile([128, NKT * 1024], BF16, tag="e")
                e_list.append(e_t0)
                s_groups(psS, e_list[0], 0, 0, 0, 2)     # units 0..5
                a_chain(psA, 2, 1)
                s_groups(psS, e_list[0], 0, 0, 2, 5)     # units 6..14
                a_chain(psA, 2, 2)
                s_groups(psS, e_list[0], 0, 0, 5, 8)     # units 15..23
                a_chain(psA, 2, 3)
                s_groups(psS, e_list[0], 0, 0, 8, 11)    # units 24..31
                for nb in range(NB):
                    a_chain(psA, 3, nb)
                a_chain(psA, 1, 0)
                e_t1 = epool.tile([128, NKT * 1024], BF16, tag="e")
                e_list.append(e_t1)
                s_groups(psS, e_list[1], 1, 0, 0, 11)    # wave 1 = (0,1)
                # waves 2 and 3 go into the PE program BEFORE the b-chains:
                # their S matmuls dribble behind the exp stream's PSUM
                # rotation, and the b/a chains fill the gaps — emitting them
                # after the chains would starve the ScalarE for ~15us at the
                # prologue/steady-state boundary.
                a_chain(psA, 0, 1)
                a_chain(psA, 1, 1)
                e_t2 = epool.tile([128, NKT * 1024], BF16, tag="e")
                e_list.append(e_t2)
                s_groups(psS, e_list[2], 0, 1, 0, 11)    # wave 2 = (1,0)
                for nt in range(NKT):
                    b_chain(psA, nt)
                a_chain(psA, 0, 2)
                a_chain(psA, 1, 2)
                a_chain(psA, 0, 3)
                a_chain(psA, 1, 3)
                psA_ctx.close()

                with (
                    tc.tile_pool(name="psU", bufs=1, space="PSUM") as psU,
                    tc.tile_pool(name="psD", bufs=1, space="PSUM") as psD,
                ):
                    for k, (nqb, pair) in enumerate(iters):
                        if k + 3 < len(iters):
                            nq2, p2 = iters[k + 3]
                            e_tn = epool.tile([128, NKT * 1024], BF16, tag="e")
                            e_list.append(e_tn)
                            s_groups(psS, e_list[k + 3], p2, nq2, 0, 11)
                        u_block(psU, pair, nqb, e_list[k])
                        dn_t = dn_block(psD, pair, nqb, e_list[k])
                        norm(rpool, rdram, dn_t, pair, nqb)
                        # projection rows, two row-tiles per iteration,
                        # deferred three iterations so the normalize
                        # latency chain (fold/recip/broadcast/muls) never
                        # gates projection matmuls
                        if k >= 3:
                            h = k - 3
                            e_block(psU, psD, opool, h * 2, h * 2 + 2)
                    for h in range(5, 8):
                        e_block(psU, psD, opool, h * 2, h * 2 + 2)

    _split_multi_waits(nc)
    return nc


_NC_CACHE = None


def _get_nc():
    global _NC_CACHE
    if _NC_CACHE is None:
        _NC_CACHE = build_core_kernel()
    return _NC_CACHE


def kernel(x, importance_weights, W_qkv, W_proj, b_proj, persistence_bias,
           _results_hook=None):
    x = np.asarray(x)
    W_qkv = np.asarray(W_qkv, dtype=np.float32)
    W_proj = np.asarray(W_proj, dtype=np.float32)
    b_proj = np.asarray(b_proj, dtype=np.float32)

    bf = ml_dtypes.bfloat16
    Q = W_qkv[:, 0:DIM]
    K = W_qkv[:, DIM:2 * DIM]
    V = W_qkv[:, 2 * DIM:3 * DIM]

    in_maps = []
    for core in range(8):
        b, g = divmod(core, 4)
        sl = slice(g * GC, (g + 1) * GC)
        in_maps.append({
            "xT": np.ascontiguousarray(x[b].T).astype(bf),
            "wqk": np.ascontiguousarray(
                np.concatenate([Q[:, sl], K[:, sl]], axis=1)).astype(bf),
            "wv": np.ascontiguousarray(V[:, sl]).astype(bf),
            "wp": np.ascontiguousarray(W_proj[sl, :]).astype(bf),
        })

    nc = _get_nc()
    res = run_bass_kernel_spmd(nc, in_maps, list(range(8)))
    if _results_hook is not None:
        _results_hook(res)

    out = np.zeros((B, N, DIM), dtype=np.float32)
    for core in range(8):
        b = core // 4
        out[b] += res.results[core]["out"].astype(np.float32)
    out += b_proj[None, None, :]
    return out
